# revision 43
# baseline (speedup 1.0000x reference)
"""Trainium2 Bass kernel for nn_MultiHeadAttention_79723182949055.

Math (per reference):
    r1 = einsum('bmp,kpd->bmkd', y, Lam_x)          # key proj
    s  = einsum('bnq,kqd->bnkd', y, Lam_y)          # query proj
    S  = einsum('bmkd,bnkd->kbmn', r1, s) + mask    # scores
    A  = softmax(S / sqrt(D), axis=m)
    w  = einsum('bmp,kpd->bmkd', y, Th_x)           # value proj
    U  = einsum('kbmn,bmkd->bnkd', A, w)            # aggregation
    out= einsum('bnkd,kqd->bnq', U, Th_y)           # out proj

Sharding: 8 cores; core c handles batch b = c//4 and heads 4*(c%4) .. +4.
Each core computes a partial out[b] (sum over its 4 heads); host sums the
4 partials per batch.

Device schedule (v2): the ACT engine's exp stream (128 tiles of
[128,1024], ~1.1us each) is the critical resource; everything else hides
under it.
  - Minimal preamble: DMA (per p-chunk: lamx, yT first n-half, lamy)
    interleaved with the (mbp0,hp0) key/query projection groups, so the
    first score matmul + exp start ~10us in.
  - Lag-1 pair pipeline over 8 (n-block, head) pairs: window w runs pair
    w's scores+exp (ACT-paced) and pair w-1's aggregation, slot-packed
    into steps 0..13; the pu accumulator drains on DVE at steps 14/15, so
    a single [65,1024] PSUM buffer suffices.
  - All remaining projection work (r1T rest, std rest, value proj w4) is
    threaded into phase-3 PE slack as 'extras' micro-ops drawing
    [128,512] PSUM accumulators from the ppo ring; marker-forced pops
    keep PE-queue emission order ahead of consumers.
  - ACT does ONLY exp (all drains/copies on DVE) -- no table reloads, no
    exp-stream interference.
  - Normalization per head-pair: colsum rows ride the aggregation (65th
    w4 column); reciprocal on DVE; one [2,128] selector matmul broadcasts
    both heads' 1/colsum rows across partitions; one DVE multiply
    normalizes both heads at once.
  - Pair 7's aggregation trails its exp stream inside window 7 (into two
    ppo-slice pu halves), so the ghost tail is only: remnant aggs, drains,
    last norms, and the final n-block's out-projection (PSUM from the
    freed score ring, drains split DVE/ACT, DMA split gpsimd/sync).
"""

import ml_dtypes
import numpy as np

import concourse.bass as bass
import concourse.bacc as bacc
import concourse.tile as tile
import concourse.mybir as mybir
from concourse.bass_utils import run_bass_kernel_spmd

F32 = mybir.dt.float32
F32R = mybir.dt.float32r
BF16 = mybir.dt.bfloat16

B, N, Q, K, P, D = 2, 2048, 1024, 16, 1024, 64
HPC = 4          # heads per core
NCORES = 8
INV_TEMP = 1.0 / 8.0    # 1/sqrt(D)

PCH = P // 128   # 8 p-chunks
MCH = N // 128   # 16 m-chunks
NBP = N // 1024  # 2 n-blocks of 1024
NPAIR = NBP * HPC

# window-w aggregation slot packing: slot s aggregates these m-chunks of
# pair w-1 (slots 0..13; steps 14/15 drain pu).
AGG_MS = [[0], [1], [2], [3], [4], [5], [6], [7], [8], [9], [10], [11],
          [12, 13], [14, 15]]
# pair order: hp0 heads first (both n-blocks), then hp1 heads -- delays
# the r1T[1]/std[2,3] projection deadlines to window 4.
PAIR_SEQ = [(0, 0), (0, 1), (1, 0), (1, 1), (0, 2), (0, 3), (1, 2), (1, 3)]


def build_program(use_mask: bool):
    nc = bacc.Bacc("TRN2", target_bir_lowering=False, debug=False,
                   num_devices=NCORES)

    # all inputs host-packed to SBUF layout: [128 partitions, chunks side
    # by side] so every DMA is fully contiguous at line rate
    yT = nc.dram_tensor("yT", [128, PCH * N], BF16, kind="ExternalInput").ap()
    lamx = nc.dram_tensor("lamx", [128, PCH * HPC * D], BF16,
                          kind="ExternalInput").ap()
    lamy = nc.dram_tensor("lamy", [128, PCH * HPC * D], BF16,
                          kind="ExternalInput").ap()
    thx = nc.dram_tensor("thx", [128, PCH * HPC * D], BF16,
                         kind="ExternalInput").ap()
    thyT = nc.dram_tensor("thyT", [128, 2 * Q], BF16,
                          kind="ExternalInput").ap()
    if use_mask:
        maskd = nc.dram_tensor("mask", [N, N], F32, kind="ExternalInput").ap()
    outp = nc.dram_tensor("outp", [N, Q], BF16, kind="ExternalOutput").ap()

    lp = nc.allow_low_precision(reason="bf16 matmul pipeline by design")
    lp.__enter__()
    with tile.TileContext(nc) as tc:
        with (
            tc.tile_pool(name="big8k", bufs=8) as bp,
            tc.tile_pool(name="wpool", bufs=1) as wp,
            tc.tile_pool(name="small", bufs=1) as sp,
            tc.tile_pool(name="epool", bufs=20) as ep,
            tc.tile_pool(name="opool", bufs=2) as op,
            tc.tile_pool(name="ps_s", bufs=2, space="PSUM") as pps,
            tc.tile_pool(name="ps_u", bufs=1, space="PSUM") as ppu,
            tc.tile_pool(name="ps_o", bufs=2, space="PSUM") as ppo,
        ):
            # ---- DMA: all inputs are host-packed to SBUF layout so each
            # transfer is one fully-contiguous big DMA (line-rate) ----
            ytl = [bp.tile([128, N], BF16, tag="bigb", bufs=8,
                           name=f"yt{p}") for p in range(PCH)]
            wxall = wp.tile([128, PCH * HPC * D], BF16, tag="wxall",
                            name="wxall")
            wyall = wp.tile([128, PCH * HPC * D], BF16, tag="wyall",
                            name="wyall")
            wtall = wp.tile([128, PCH * HPC * D], BF16, tag="wtall",
                            name="wtall")
            thytall = wp.tile([128, 2 * Q], BF16, tag="thyt", name="thyt")
            # one contiguous line-rate DMA per p-chunk / weight tensor;
            # first p-chunks and first weight halves lead their queues
            HW = PCH * HPC * D // 2
            nc.gpsimd.dma_start(out=ytl[0], in_=yT[:, 0:N])
            nc.scalar.dma_start(out=ytl[1], in_=yT[:, N:2 * N])
            nc.sync.dma_start(out=wxall[:, 0:HW], in_=lamx[:, 0:HW])
            nc.sync.dma_start(out=wyall[:, 0:HW], in_=lamy[:, 0:HW])
            for p in range(2, PCH):
                dq = nc.gpsimd if p % 2 == 0 else nc.scalar
                dq.dma_start(out=ytl[p], in_=yT[:, p * N:(p + 1) * N])
            nc.sync.dma_start(out=wxall[:, HW:2 * HW], in_=lamx[:, HW:2 * HW])
            nc.sync.dma_start(out=wyall[:, HW:2 * HW], in_=lamy[:, HW:2 * HW])
            nc.sync.dma_start(out=wtall, in_=thx)
            nc.sync.dma_start(out=thytall, in_=thyT)

            def yt(p):
                return ytl[p]

            def wsl(wall, p, lo, hi):
                return wall[:, p * HPC * D + lo:p * HPC * D + hi]

            def thyt(j2, lo, hi):
                return thytall[:, j2 * Q + lo:j2 * Q + hi]

            # ---- constants (DVE is idle while the first DMAs land) ----
            wdum = sp.tile([128, 512], BF16, tag="wdum", name="wdum")
            nc.vector.memset(wdum.bitcast(F32), 0.0)
            onescol = sp.tile([128, 1], F32, tag="onescol", name="onescol")
            nc.vector.memset(onescol, 1.0)
            ones_row = sp.tile([1, 64], F32R, tag="ones_row", name="ones_row")
            nc.vector.memset(ones_row.bitcast(F32), 1.0)

            std = [bp.tile([128, N], BF16, tag="big", name=f"std{j}")
                   for j in range(HPC)]
            for j in range(HPC):
                zlo, zhi = (64, 128) if j % 2 == 0 else (0, 64)
                nc.vector.memset(std[j][zlo:zhi, :].bitcast(F32), 0.0)

            w4 = [sp.tile([128, HPC * 65], BF16, tag=f"w4_{m}", name=f"w4_{m}")
                  for m in range(MCH)]

            r1T = [bp.tile([128, N], BF16, tag="big", name=f"r1T{hp}")
                   for hp in range(2)]
            uta = [bp.tile([128, N], BF16, tag="big", name=f"uta{j2}")
                   for j2 in range(2)]
            # paired colsum sets: pair (2k, 2k+1) shares set k%2; even
            # pair's colsum on row 0, odd pair's on row 32 (32-aligned).
            # memset 1.0 keeps unused rows finite through the reciprocal.
            csr = [sp.tile([64, 1024], F32, tag=f"csr{s}", name=f"csr{s}")
                   for s in range(2)]
            inv0 = [sp.tile([64, 1024], F32, tag=f"inv{s}", name=f"inv{s}")
                    for s in range(2)]
            invr = [sp.tile([64, 1024], F32R, tag=f"invr{s}", name=f"invr{s}")
                    for s in range(2)]
            for s in range(2):
                nc.vector.memset(csr[s], 1.0)
            # selector: output rows 0..63 get csr row 0, rows 64..127 row 32
            ones2sel = sp.tile([64, 128], F32R, tag="ones2sel",
                               name="ones2sel")
            nc.vector.memset(ones2sel.bitcast(F32), 0.0)
            nc.vector.memset(ones2sel[0:1, 0:64].bitcast(F32), 1.0)
            nc.vector.memset(ones2sel[32:33, 64:128].bitcast(F32), 1.0)

            # warm-up: garbage matmuls ramp the PE clock while DMAs land
            def emit_dummies(n, pool):
                for _ in range(n):
                    if pool is ppo:
                        dps = ppo.tile([128, 512], F32, tag="po", name="dps")
                    else:
                        dps = pps.tile([128, 1024], F32, tag="ps",
                                       name="dps")[:, 0:512]
                    nc.tensor.matmul(dps, wdum[:, 0:128], wdum,
                                     start=True, stop=True)

            emit_dummies(14, ppo)

            # ---- preamble: 1a+1b (mbp0, hp0), p-interleaved with DMA ----
            acc_a = pps.tile([128, 1024], F32, tag="ps", name="acc_a")
            acc_b = pps.tile([128, 1024], F32, tag="ps", name="acc_b")
            for p in range(PCH):
                for mh in range(2):
                    nc.tensor.matmul(
                        acc_a[:, mh * 512:(mh + 1) * 512],
                        wsl(wxall, p, 0, 128),
                        yt(p)[:, mh * 512:(mh + 1) * 512],
                        start=(p == 0), stop=(p == PCH - 1))
                for mh in range(2):
                    nc.tensor.matmul(
                        acc_b[:, mh * 512:(mh + 1) * 512],
                        wsl(wyall, p, 0, 128),
                        yt(p)[:, mh * 512:(mh + 1) * 512],
                        start=(p == 0), stop=(p == PCH - 1))
                if p < PCH - 1:
                    emit_dummies(2, ppo)
            # copies ordered so scores(0, m=0) unblocks as early as possible
            nc.vector.tensor_copy(out=r1T[0][:, 0:512], in_=acc_a[:, 0:512])
            nc.vector.tensor_copy(out=std[0][0:64, 0:1024], in_=acc_b[0:64, :])
            nc.vector.tensor_copy(out=r1T[0][:, 512:1024],
                                  in_=acc_a[:, 512:1024])
            nc.vector.tensor_copy(out=std[1][64:128, 0:1024],
                                  in_=acc_b[64:128, :])
            # w4 ones-columns: DVE is otherwise idle here; needed by w1 aggs
            for m in range(MCH):
                nc.vector.tensor_copy(
                    out=w4[m],
                    in_=onescol[:, 0:1].broadcast_to((128, HPC * 65)))

            # ---- extras machinery ----
            done = set()
            done.add("r1T0_0_0")
            done.add("r1T0_0_1")
            done.add("std_hp0_n0")
            ex_q = []
            qi = [0]

            def push_1a_half(hp, mbp, mh, marker=None):
                cell = {}
                for p in range(PCH):
                    def mm_i(p=p, hp=hp, mbp=mbp, mh=mh):
                        if p == 0:
                            cell["acc"] = ppo.tile([128, 512], F32, tag="po",
                                                   name="xacc")
                        nc.tensor.matmul(
                            cell["acc"],
                            wsl(wxall, p, hp * 128, (hp + 1) * 128),
                            yt(p)[:, (mbp * 2 + mh) * 512:
                                   (mbp * 2 + mh + 1) * 512],
                            start=(p == 0), stop=(p == PCH - 1))
                    ex_q.append(mm_i)

                def fin(hp=hp, mbp=mbp, mh=mh, marker=marker):
                    sl = slice((mbp * 2 + mh) * 512, (mbp * 2 + mh + 1) * 512)
                    nc.vector.tensor_copy(out=r1T[hp][:, sl], in_=cell["acc"])
                    done.add(f"r1T{hp}_{mbp}_{mh}")
                    if marker:
                        done.add(marker)
                ex_q.append(fin)

            def push_1b_half(hp, nbp, mh, marker=None):
                cell = {}
                for p in range(PCH):
                    def mm_i(p=p, hp=hp, nbp=nbp, mh=mh):
                        if p == 0:
                            cell["acc"] = ppo.tile([128, 512], F32, tag="po",
                                                   name="sacc")
                        nc.tensor.matmul(
                            cell["acc"],
                            wsl(wyall, p, hp * 128, (hp + 1) * 128),
                            yt(p)[:, (nbp * 2 + mh) * 512:
                                   (nbp * 2 + mh + 1) * 512],
                            start=(p == 0), stop=(p == PCH - 1))
                    ex_q.append(mm_i)

                def fin(hp=hp, nbp=nbp, mh=mh, marker=marker):
                    sl = slice((nbp * 2 + mh) * 512, (nbp * 2 + mh + 1) * 512)
                    nc.vector.tensor_copy(out=std[2 * hp][0:64, sl],
                                          in_=cell["acc"][0:64, :])
                    nc.vector.tensor_copy(out=std[2 * hp + 1][64:128, sl],
                                          in_=cell["acc"][64:128, :])
                    if marker:
                        done.add(marker)
                ex_q.append(fin)

            def push_ph2(m):
                cell = {}
                for p in range(PCH):
                    def mm_i(p=p, m=m):
                        if p == 0:
                            cell["acc"] = ppo.tile([128, 512], F32, tag="po",
                                                   name="wacc")
                        nc.tensor.matmul(
                            cell["acc"][:, 0:HPC * D],
                            yt(p)[:, m * 128:(m + 1) * 128],
                            wsl(wtall, p, 0, HPC * D),
                            start=(p == 0), stop=(p == PCH - 1))
                    ex_q.append(mm_i)

                def fin(m=m):
                    src3 = cell["acc"][:, 0:HPC * D].rearrange(
                        "p (h c) -> p h c", h=HPC)
                    dst3 = w4[m].rearrange("p (h c) -> p h c", h=HPC,
                                           c=65)[:, :, 0:64]
                    nc.vector.tensor_copy(out=dst3, in_=src3)
                    done.add(f"w4_{m}")
                ex_q.append(fin)

            # order matters: consumers force-pop up to their marker
            push_1a_half(0, 1, 0)
            push_1a_half(0, 1, 1)
            for m in range(MCH):
                push_ph2(m)
            push_1b_half(0, 1, 0)
            push_1b_half(0, 1, 1, marker="std_hp0_n1")
            push_1a_half(1, 0, 0)
            push_1a_half(1, 0, 1)
            push_1a_half(1, 1, 0)
            push_1a_half(1, 1, 1)
            push_1b_half(1, 0, 0)
            push_1b_half(1, 0, 1, marker="std_hp1_n0")
            push_1b_half(1, 1, 0)
            push_1b_half(1, 1, 1, marker="std_hp1_n1")

            def pop_extras(k):
                while k > 0 and qi[0] < len(ex_q):
                    ex_q[qi[0]]()
                    qi[0] += 1
                    k -= 1

            def force(marker):
                while marker not in done and qi[0] < len(ex_q):
                    ex_q[qi[0]]()
                    qi[0] += 1

            # ---- misc queue (norms + mid out-proj), popped 1 per step ----
            misc_q = []

            def pop_misc():
                if misc_q:
                    misc_q.pop(0)()

            def push_norm(opair, use_pps=False):
                """normalize the (opair-1, opair) head-pair's uta block."""
                s = (opair // 2) % 2
                nbp, j = PAIR_SEQ[opair]
                hp = j // 2

                def t_recip(s=s):
                    nc.vector.reciprocal_approx_fast(out=inv0[s], in_=csr[s])
                    nc.vector.tensor_copy(out=invr[s], in_=inv0[s])
                misc_q.append(t_recip)
                for h in range(2):
                    def t_bc(s=s, nbp=nbp, hp=hp, h=h, use_pps=use_pps):
                        if use_pps:
                            pb = pps.tile([128, 1024], F32, tag="ps",
                                          name="pb")[:, 0:512]
                        else:
                            pb = ppo.tile([128, 512], F32, tag="po",
                                          name="pb")
                        nc.tensor.matmul(
                            pb, ones2sel,
                            invr[s][:, h * 512:(h + 1) * 512],
                            start=True, stop=True)
                        nsl = slice(nbp * 1024 + h * 512,
                                    nbp * 1024 + (h + 1) * 512)
                        nc.vector.tensor_mul(uta[hp][:, nsl],
                                             uta[hp][:, nsl], pb)
                    misc_q.append(t_bc)

            osb_cache = {}

            def push_outproj(nbp):
                for nch in range(nbp * 8, nbp * 8 + 8):
                    for qb in range(2):
                        def t_op(nch=nch, qb=qb):
                            po = ppo.tile([128, 512], F32, tag="po",
                                          name="po")
                            for j2 in range(2):
                                nc.tensor.matmul(
                                    po,
                                    uta[j2][:, nch * 128:(nch + 1) * 128],
                                    thyt(j2, qb * 512, (qb + 1) * 512),
                                    start=(j2 == 0), stop=(j2 == 1))
                            if qb == 0:
                                osb_cache[nch] = op.tile(
                                    [128, 1024], BF16, tag="osb", bufs=4,
                                    name="osb")
                            osb = osb_cache[nch]
                            nc.vector.tensor_copy(
                                out=osb[:, qb * 512:(qb + 1) * 512], in_=po)
                            if qb == 1:
                                nc.gpsimd.dma_start(
                                    out=outp[nch * 128:(nch + 1) * 128, :],
                                    in_=osb)
                        misc_q.append(t_op)

            # ---- emit helpers ----
            e_t = {}

            def emit_scores(pair, m):
                nbp, j = PAIR_SEQ[pair]
                hp, n0 = j // 2, nbp * 1024
                force(f"r1T{hp}_{m // 8}_{(m % 8) // 4}")
                force(f"std_hp{hp}_n{nbp}")
                pst = pps.tile([128, 1024], F32, tag="ps", name="ps")
                for h in range(2):
                    nc.tensor.matmul(
                        pst[:, h * 512:(h + 1) * 512],
                        r1T[hp][:, m * 128:(m + 1) * 128],
                        std[j][:, n0 + h * 512:n0 + (h + 1) * 512],
                        start=True, stop=True)
                if use_mask:
                    mt = op.tile([128, 1024], F32, tag="mt", name="mt")
                    nc.gpsimd.dma_start(
                        out=mt, in_=maskd[m * 128:(m + 1) * 128, n0:n0 + 1024])
                    nc.vector.tensor_add(pst, pst, mt)
                et = ep.tile([128, 1024], BF16, tag="e", name="e")
                e_t[(pair, m)] = et
                nc.scalar.activation(
                    out=et, in_=pst, func=mybir.ActivationFunctionType.Exp,
                    scale=INV_TEMP)

            def emit_agg(pair, ms, pu):
                j = PAIR_SEQ[pair][1]
                for m in ms:
                    force(f"w4_{m}")
                    for h in range(2):
                        nc.tensor.matmul(
                            pu[:, h * 512:(h + 1) * 512],
                            w4[m][:, j * 65:j * 65 + 65],
                            e_t[(pair, m)][:, h * 512:(h + 1) * 512],
                            start=(m == 0), stop=(m == MCH - 1))
                    e_t.pop((pair, m))

            def emit_drain(pair, pu):
                nbp, j = PAIR_SEQ[pair]
                hp, po, n0 = j // 2, 64 * (j % 2), nbp * 1024
                s, row = (pair // 2) % 2, 32 * (pair % 2)
                nc.vector.tensor_copy(out=uta[hp][po:po + 64, n0:n0 + 1024],
                                      in_=pu[0:64, :])
                nc.vector.tensor_copy(out=csr[s][row:row + 1, :],
                                      in_=pu[64:65, :])

            # ---- windows ----
            pu_cur = [None]     # pair w-1's [65,1024] pu (ppu pool)
            pu7h = [None, None]  # pair 7 trailing pu halves (ppo slices)
            for w in range(NPAIR):
                nbp, j = PAIR_SEQ[w]
                for step in range(16):
                    if w == NPAIR - 1 and step == 0:
                        # ppo must be free for the trailing pu halves
                        pop_extras(len(ex_q))
                        while misc_q:
                            pop_misc()
                    has_agg = False
                    if w >= 1 and step < len(AGG_MS):
                        if step == 0:
                            pu_cur[0] = ppu.tile([65, 1024], F32, tag="pu",
                                                 name="pu")
                        emit_agg(w - 1, AGG_MS[step], pu_cur[0])
                        has_agg = True
                    emit_scores(w, step)
                    if w == NPAIR - 1 and step >= 2:
                        # trailing aggregation of the last pair
                        m = step - 2
                        if m == 0:
                            pu7h[0] = ppo.tile([128, 512], F32, tag="po",
                                               name="pu7a")[0:65, :]
                            pu7h[1] = ppo.tile([128, 512], F32, tag="po",
                                               name="pu7b")[0:65, :]
                        force(f"w4_{m}")
                        for h in range(2):
                            nc.tensor.matmul(
                                pu7h[h],
                                w4[m][:, j * 65:j * 65 + 65],
                                e_t[(w, m)][:, h * 512:(h + 1) * 512],
                                start=(m == 0), stop=(m == MCH - 1))
                        e_t.pop((w, m))
                    if w >= 1 and step == 14:
                        emit_drain(w - 1, pu_cur[0])
                    if w >= 3:
                        pop_misc()
                    if step >= 14:
                        pop_extras(2)
                    elif has_agg and len(AGG_MS[step]) > 1:
                        pass  # double-agg slots are already over budget
                    else:
                        pop_extras(3 if not has_agg else (2 if w < 4 else 1))
                # norm thunks become available after the odd pair's drain
                if w >= 1 and w - 1 <= 5 and (w - 1) % 2 == 1:
                    push_norm(w - 1)
                    if w - 1 == 5:
                        push_outproj(0)

            # ---- ghost tail ----
            pop_extras(len(ex_q))
            while misc_q:
                pop_misc()
            # pair 7 trailing remnant: m=14,15 (steps 2..15 covered 0..13)
            j7 = (NPAIR - 1) % HPC
            for m in range(14, MCH):
                for h in range(2):
                    nc.tensor.matmul(
                        pu7h[h],
                        w4[m][:, j7 * 65:j7 * 65 + 65],
                        e_t[(NPAIR - 1, m)][:, h * 512:(h + 1) * 512],
                        start=(m == 0), stop=(m == MCH - 1))
                e_t.pop((NPAIR - 1, m))
            # --- ghost norm + out-projection, pipelined by n-half ---
            # pairs 6,7 share colsum set 1: pair 6's row 0 was written by
            # its w7.14 drain; pair 7's row 32 comes from the pu7h halves
            for h in range(2):
                nc.vector.tensor_copy(
                    out=csr[1][32:33, h * 512:(h + 1) * 512],
                    in_=pu7h[h][64:65, :])
            nc.vector.reciprocal_approx_fast(out=inv0[1], in_=csr[1])
            nc.vector.tensor_copy(out=invr[1], in_=inv0[1])
            emit_dummies(6, pps)
            # pair-7 uta rows drain on the idle ACT
            for h in range(2):
                sl = slice(1024 + h * 512, 1024 + (h + 1) * 512)
                nc.scalar.copy(out=uta[1][64:128, sl], in_=pu7h[h][0:64, :])

            def ghost_norm(h):
                pb = pps.tile([128, 1024], F32, tag="ps",
                              name="pb")[:, 0:512]
                nc.tensor.matmul(pb, ones2sel,
                                 invr[1][:, h * 512:(h + 1) * 512],
                                 start=True, stop=True)
                nsl = slice(1024 + h * 512, 1024 + (h + 1) * 512)
                nc.vector.tensor_mul(uta[1][:, nsl], uta[1][:, nsl], pb)

            for h in range(2):
                ghost_norm(h)
                if h == 0:
                    emit_dummies(2, pps)
                # out-projection for this n-half (nch 8+4h .. 11+4h)
                for i, (nch, qb) in enumerate(
                        (nc_, qb_) for nc_ in range(8 + 4 * h, 12 + 4 * h)
                        for qb_ in range(2)):
                    po = pps.tile([128, 1024], F32, tag="ps",
                                  name="pof")[:, 0:512]
                    for j2 in range(2):
                        nc.tensor.matmul(
                            po,
                            uta[j2][:, nch * 128:(nch + 1) * 128],
                            thyt(j2, qb * 512, (qb + 1) * 512),
                            start=(j2 == 0), stop=(j2 == 1))
                    if qb == 0:
                        osb_cache[nch] = op.tile([128, 1024], BF16,
                                                 tag="osb", bufs=4,
                                                 name="osbf")
                    osb = osb_cache[nch]
                    if i % 2 == 0:
                        nc.vector.tensor_copy(
                            out=osb[:, qb * 512:(qb + 1) * 512], in_=po)
                    else:
                        nc.scalar.copy(out=osb[:, qb * 512:(qb + 1) * 512],
                                       in_=po)
                    if qb == 1:
                        dq = (nc.gpsimd, nc.sync, nc.scalar)[nch % 3]
                        dq.dma_start(
                            out=outp[nch * 128:(nch + 1) * 128, :], in_=osb)
    lp.__exit__(None, None, None)
    nc.compile()
    return nc


_PROG_CACHE = {}


def _get_program(use_mask: bool):
    if use_mask not in _PROG_CACHE:
        _PROG_CACHE[use_mask] = build_program(use_mask)
    return _PROG_CACHE[use_mask]


def _pack(a):
    """[(c*128), k] -> [128, c*k]: p-chunks side by side, SBUF layout."""
    c = a.shape[0] // 128
    return np.ascontiguousarray(
        a.reshape(c, 128, a.shape[1]).transpose(1, 0, 2).reshape(128, -1)
    ).astype(ml_dtypes.bfloat16)


def make_in_maps(y_prime, mask, Lam_x, Lam_y, Th_x, Th_y, use_mask):
    in_maps = []
    for c in range(NCORES):
        b = c // 4
        heads = [4 * (c % 4) + j for j in range(HPC)]
        m = {
            "yT": _pack(y_prime[b].T),
            "lamx": _pack(Lam_x[heads].transpose(1, 0, 2).reshape(P, HPC * D)),
            "lamy": _pack(Lam_y[heads].transpose(1, 0, 2).reshape(P, HPC * D)),
            "thx": _pack(Th_x[heads].transpose(1, 0, 2).reshape(P, HPC * D)),
            "thyT": _pack(Th_y[heads].transpose(0, 2, 1).reshape(HPC * D, Q)),
        }
        if use_mask:
            m["mask"] = np.ascontiguousarray(mask).astype(np.float32)
        in_maps.append(m)
    return in_maps


def kernel(y_prime, mask, Lam_x, Lam_y, Th_x, Th_y, _trace=False):
    y_prime = np.asarray(y_prime, dtype=np.float32)
    mask = np.asarray(mask, dtype=np.float32)
    Lam_x = np.asarray(Lam_x, dtype=np.float32)
    Lam_y = np.asarray(Lam_y, dtype=np.float32)
    Th_x = np.asarray(Th_x, dtype=np.float32)
    Th_y = np.asarray(Th_y, dtype=np.float32)

    use_mask = bool(np.any(mask))
    nc = _get_program(use_mask)
    in_maps = make_in_maps(y_prime, mask, Lam_x, Lam_y, Th_x, Th_y, use_mask)
    r = run_bass_kernel_spmd(nc, in_maps, core_ids=list(range(NCORES)),
                             trace=_trace)
    out = np.zeros((B, N, Q), dtype=np.float32)
    for c in range(NCORES):
        out[c // 4] += np.asarray(r.results[c]["outp"]).astype(np.float32)
    if _trace:
        kernel.last_results = r
    return out


# revision 44
# speedup vs baseline: 1.1866x; 1.1866x over previous
"""Trainium2 Bass kernel for nn_MultiHeadAttention_79723182949055.

Math (per reference):
    r1 = einsum('bmp,kpd->bmkd', y, Lam_x)          # key proj
    s  = einsum('bnq,kqd->bnkd', y, Lam_y)          # query proj
    S  = einsum('bmkd,bnkd->kbmn', r1, s) + mask    # scores
    A  = softmax(S / sqrt(D), axis=m)
    w  = einsum('bmp,kpd->bmkd', y, Th_x)           # value proj
    U  = einsum('kbmn,bmkd->bnkd', A, w)            # aggregation
    out= einsum('bnkd,kqd->bnq', U, Th_y)           # out proj

Sharding: 8 cores; core c handles batch b = c//4 and heads 4*(c%4) .. +4.
Each core computes a partial out[b] (sum over its 4 heads); host sums the
4 partials per batch.

Device schedule (v2): the ACT engine's exp stream (128 tiles of
[128,1024], ~1.1us each) is the critical resource; everything else hides
under it.
  - Minimal preamble: DMA (per p-chunk: lamx, yT first n-half, lamy)
    interleaved with the (mbp0,hp0) key/query projection groups, so the
    first score matmul + exp start ~10us in.
  - Lag-1 pair pipeline over 8 (n-block, head) pairs: window w runs pair
    w's scores+exp (ACT-paced) and pair w-1's aggregation, slot-packed
    into steps 0..13; the pu accumulator drains on DVE at steps 14/15, so
    a single [65,1024] PSUM buffer suffices.
  - All remaining projection work (r1T rest, std rest, value proj w4) is
    threaded into phase-3 PE slack as 'extras' micro-ops drawing
    [128,512] PSUM accumulators from the ppo ring; marker-forced pops
    keep PE-queue emission order ahead of consumers.
  - ACT does ONLY exp (all drains/copies on DVE) -- no table reloads, no
    exp-stream interference.
  - Normalization per head-pair: colsum rows ride the aggregation (65th
    w4 column); reciprocal on DVE; one [2,128] selector matmul broadcasts
    both heads' 1/colsum rows across partitions; one DVE multiply
    normalizes both heads at once.
  - Pair 7's aggregation trails its exp stream inside window 7 (into two
    ppo-slice pu halves), so the ghost tail is only: remnant aggs, drains,
    last norms, and the final n-block's out-projection (PSUM from the
    freed score ring, drains split DVE/ACT, DMA split gpsimd/sync).
"""

import ml_dtypes
import numpy as np

import concourse.bass as bass
import concourse.bacc as bacc
import concourse.tile as tile
import concourse.mybir as mybir
from concourse.bass_utils import run_bass_kernel_spmd

F32 = mybir.dt.float32
F32R = mybir.dt.float32r
BF16 = mybir.dt.bfloat16

B, N, Q, K, P, D = 2, 2048, 1024, 16, 1024, 64
HPC = 4          # heads per core
NCORES = 8
INV_TEMP = 1.0 / 8.0    # 1/sqrt(D)

PCH = P // 128   # 8 p-chunks
MCH = N // 128   # 16 m-chunks
NBP = N // 1024  # 2 n-blocks of 1024
NPAIR = NBP * HPC

# window-w aggregation slot packing: slot s aggregates these m-chunks of
# pair w-1 (slots 0..13; steps 14/15 drain pu).
AGG_MS = [[0], [1], [2], [3], [4], [5], [6], [7], [8], [9], [10], [11],
          [12, 13], [14, 15]]
# pair order: hp0 heads first (both n-blocks), then hp1 heads -- delays
# the r1T[1]/std[2,3] projection deadlines to window 4.
PAIR_SEQ = [(0, 0), (0, 1), (1, 0), (1, 1), (0, 2), (0, 3), (1, 2), (1, 3)]


def build_program(use_mask: bool):
    nc = bacc.Bacc("TRN2", target_bir_lowering=False, debug=False,
                   num_devices=NCORES)

    # all inputs host-packed to SBUF layout: [128 partitions, chunks side
    # by side] so every DMA is fully contiguous at line rate
    yT = nc.dram_tensor("yT", [128, PCH * N], BF16, kind="ExternalInput").ap()
    lamx = nc.dram_tensor("lamx", [128, PCH * HPC * D], BF16,
                          kind="ExternalInput").ap()
    lamy = nc.dram_tensor("lamy", [128, PCH * HPC * D], BF16,
                          kind="ExternalInput").ap()
    thx = nc.dram_tensor("thx", [128, PCH * HPC * D], BF16,
                         kind="ExternalInput").ap()
    thyT = nc.dram_tensor("thyT", [128, 2 * Q], BF16,
                          kind="ExternalInput").ap()
    if use_mask:
        maskd = nc.dram_tensor("mask", [N, N], F32, kind="ExternalInput").ap()
    outp = nc.dram_tensor("outp", [N, Q], BF16, kind="ExternalOutput").ap()

    lp = nc.allow_low_precision(reason="bf16 matmul pipeline by design")
    lp.__enter__()
    with tile.TileContext(nc) as tc:
        with (
            tc.tile_pool(name="big8k", bufs=8) as bp,
            tc.tile_pool(name="wpool", bufs=1) as wp,
            tc.tile_pool(name="small", bufs=1) as sp,
            tc.tile_pool(name="epool", bufs=20) as ep,
            tc.tile_pool(name="opool", bufs=2) as op,
            tc.tile_pool(name="ps_s", bufs=2, space="PSUM") as pps,
            tc.tile_pool(name="ps_u", bufs=1, space="PSUM") as ppu,
            tc.tile_pool(name="ps_o", bufs=2, space="PSUM") as ppo,
        ):
            # ---- DMA: all inputs are host-packed to SBUF layout so each
            # transfer is one fully-contiguous big DMA (line-rate) ----
            ytl = [bp.tile([128, N], BF16, tag="bigb", bufs=8,
                           name=f"yt{p}") for p in range(PCH)]
            wxall = wp.tile([128, PCH * HPC * D], BF16, tag="wxall",
                            name="wxall")
            wyall = wp.tile([128, PCH * HPC * D], BF16, tag="wyall",
                            name="wyall")
            wtall = wp.tile([128, PCH * HPC * D], BF16, tag="wtall",
                            name="wtall")
            thytall = wp.tile([128, 2 * Q], BF16, tag="thyt", name="thyt")
            # one contiguous line-rate DMA per p-chunk / weight tensor;
            # first p-chunks and first weight halves lead their queues
            HW = PCH * HPC * D // 2
            nc.gpsimd.dma_start(out=ytl[0], in_=yT[:, 0:N])
            nc.scalar.dma_start(out=ytl[1], in_=yT[:, N:2 * N])
            nc.sync.dma_start(out=wxall[:, 0:HW], in_=lamx[:, 0:HW])
            nc.sync.dma_start(out=wyall[:, 0:HW], in_=lamy[:, 0:HW])
            for p in range(2, PCH):
                dq = nc.gpsimd if p % 2 == 0 else nc.scalar
                dq.dma_start(out=ytl[p], in_=yT[:, p * N:(p + 1) * N])
            nc.sync.dma_start(out=wxall[:, HW:2 * HW], in_=lamx[:, HW:2 * HW])
            nc.sync.dma_start(out=wyall[:, HW:2 * HW], in_=lamy[:, HW:2 * HW])
            nc.sync.dma_start(out=wtall, in_=thx)
            nc.sync.dma_start(out=thytall, in_=thyT)

            def yt(p):
                return ytl[p]

            def wsl(wall, p, lo, hi):
                return wall[:, p * HPC * D + lo:p * HPC * D + hi]

            def thyt(j2, lo, hi):
                return thytall[:, j2 * Q + lo:j2 * Q + hi]

            # ---- constants (DVE is idle while the first DMAs land) ----
            wdum = sp.tile([128, 512], BF16, tag="wdum", name="wdum")
            nc.vector.memset(wdum.bitcast(F32), 0.0)
            onescol = sp.tile([128, 1], F32, tag="onescol", name="onescol")
            nc.vector.memset(onescol, 1.0)
            ones_row = sp.tile([1, 64], F32R, tag="ones_row", name="ones_row")
            nc.vector.memset(ones_row.bitcast(F32), 1.0)

            std = [bp.tile([128, N], BF16, tag="big", name=f"std{j}")
                   for j in range(HPC)]
            for j in range(HPC):
                zlo, zhi = (64, 128) if j % 2 == 0 else (0, 64)
                nc.vector.memset(std[j][zlo:zhi, :].bitcast(F32), 0.0)

            w4 = [sp.tile([128, HPC * 65], BF16, tag=f"w4_{m}", name=f"w4_{m}")
                  for m in range(MCH)]

            r1T = [bp.tile([128, N], BF16, tag="big", name=f"r1T{hp}")
                   for hp in range(2)]
            uta = [bp.tile([128, N], BF16, tag="big", name=f"uta{j2}")
                   for j2 in range(2)]
            # paired colsum sets: pair (2k, 2k+1) shares set k%2; even
            # pair's colsum on row 0, odd pair's on row 32 (32-aligned).
            # memset 1.0 keeps unused rows finite through the reciprocal.
            csr = [sp.tile([64, 1024], F32, tag=f"csr{s}", name=f"csr{s}")
                   for s in range(2)]
            inv0 = [sp.tile([64, 1024], F32, tag=f"inv{s}", name=f"inv{s}")
                    for s in range(2)]
            invr = [sp.tile([64, 1024], F32R, tag=f"invr{s}", name=f"invr{s}")
                    for s in range(2)]
            for s in range(2):
                nc.vector.memset(csr[s], 1.0)
            # selector: output rows 0..63 get csr row 0, rows 64..127 row 32
            ones2sel = sp.tile([64, 128], F32R, tag="ones2sel",
                               name="ones2sel")
            nc.vector.memset(ones2sel.bitcast(F32), 0.0)
            nc.vector.memset(ones2sel[0:1, 0:64].bitcast(F32), 1.0)
            nc.vector.memset(ones2sel[32:33, 64:128].bitcast(F32), 1.0)

            # warm-up: garbage matmuls ramp the PE clock while DMAs land
            def emit_dummies(n, pool):
                for _ in range(n):
                    if pool is ppo:
                        dps = ppo.tile([128, 512], F32, tag="po", name="dps")
                    else:
                        dps = pps.tile([128, 1024], F32, tag="ps",
                                       name="dps")[:, 0:512]
                    nc.tensor.matmul(dps, wdum[:, 0:128], wdum,
                                     start=True, stop=True)

            emit_dummies(14, ppo)

            # ---- preamble: 1a+1b (mbp0, hp0), p-interleaved with DMA ----
            acc_a = pps.tile([128, 1024], F32, tag="ps", name="acc_a")
            acc_b = pps.tile([128, 1024], F32, tag="ps", name="acc_b")
            for p in range(PCH):
                for mh in range(2):
                    nc.tensor.matmul(
                        acc_a[:, mh * 512:(mh + 1) * 512],
                        wsl(wxall, p, 0, 128),
                        yt(p)[:, mh * 512:(mh + 1) * 512],
                        start=(p == 0), stop=(p == PCH - 1))
                for mh in range(2):
                    nc.tensor.matmul(
                        acc_b[:, mh * 512:(mh + 1) * 512],
                        wsl(wyall, p, 0, 128),
                        yt(p)[:, mh * 512:(mh + 1) * 512],
                        start=(p == 0), stop=(p == PCH - 1))
            # copies ordered so scores(0, m=0) unblocks as early as possible
            nc.vector.tensor_copy(out=r1T[0][:, 0:512], in_=acc_a[:, 0:512])
            nc.vector.tensor_copy(out=std[0][0:64, 0:1024], in_=acc_b[0:64, :])
            nc.vector.tensor_copy(out=r1T[0][:, 512:1024],
                                  in_=acc_a[:, 512:1024])
            nc.vector.tensor_copy(out=std[1][64:128, 0:1024],
                                  in_=acc_b[64:128, :])
            # w4 ones-columns: DVE is otherwise idle here; needed by w1 aggs
            for m in range(MCH):
                nc.vector.tensor_copy(
                    out=w4[m],
                    in_=onescol[:, 0:1].broadcast_to((128, HPC * 65)))

            # ---- extras machinery ----
            done = set()
            done.add("r1T0_0_0")
            done.add("r1T0_0_1")
            done.add("std_hp0_n0")
            ex_q = []
            qi = [0]

            def push_1a_half(hp, mbp, mh, marker=None):
                cell = {}
                for p in range(PCH):
                    def mm_i(p=p, hp=hp, mbp=mbp, mh=mh):
                        if p == 0:
                            cell["acc"] = ppo.tile([128, 512], F32, tag="po",
                                                   name="xacc")
                        nc.tensor.matmul(
                            cell["acc"],
                            wsl(wxall, p, hp * 128, (hp + 1) * 128),
                            yt(p)[:, (mbp * 2 + mh) * 512:
                                   (mbp * 2 + mh + 1) * 512],
                            start=(p == 0), stop=(p == PCH - 1))
                    ex_q.append(mm_i)

                def fin(hp=hp, mbp=mbp, mh=mh, marker=marker):
                    sl = slice((mbp * 2 + mh) * 512, (mbp * 2 + mh + 1) * 512)
                    nc.vector.tensor_copy(out=r1T[hp][:, sl], in_=cell["acc"])
                    done.add(f"r1T{hp}_{mbp}_{mh}")
                    if marker:
                        done.add(marker)
                ex_q.append(fin)

            def push_1b_half(hp, nbp, mh, marker=None):
                cell = {}
                for p in range(PCH):
                    def mm_i(p=p, hp=hp, nbp=nbp, mh=mh):
                        if p == 0:
                            cell["acc"] = ppo.tile([128, 512], F32, tag="po",
                                                   name="sacc")
                        nc.tensor.matmul(
                            cell["acc"],
                            wsl(wyall, p, hp * 128, (hp + 1) * 128),
                            yt(p)[:, (nbp * 2 + mh) * 512:
                                   (nbp * 2 + mh + 1) * 512],
                            start=(p == 0), stop=(p == PCH - 1))
                    ex_q.append(mm_i)

                def fin(hp=hp, nbp=nbp, mh=mh, marker=marker):
                    sl = slice((nbp * 2 + mh) * 512, (nbp * 2 + mh + 1) * 512)
                    nc.vector.tensor_copy(out=std[2 * hp][0:64, sl],
                                          in_=cell["acc"][0:64, :])
                    nc.vector.tensor_copy(out=std[2 * hp + 1][64:128, sl],
                                          in_=cell["acc"][64:128, :])
                    if marker:
                        done.add(marker)
                ex_q.append(fin)

            def push_ph2(m):
                cell = {}
                for p in range(PCH):
                    def mm_i(p=p, m=m):
                        if p == 0:
                            cell["acc"] = ppo.tile([128, 512], F32, tag="po",
                                                   name="wacc")
                        nc.tensor.matmul(
                            cell["acc"][:, 0:HPC * D],
                            yt(p)[:, m * 128:(m + 1) * 128],
                            wsl(wtall, p, 0, HPC * D),
                            start=(p == 0), stop=(p == PCH - 1))
                    ex_q.append(mm_i)

                def fin(m=m):
                    src3 = cell["acc"][:, 0:HPC * D].rearrange(
                        "p (h c) -> p h c", h=HPC)
                    dst3 = w4[m].rearrange("p (h c) -> p h c", h=HPC,
                                           c=65)[:, :, 0:64]
                    nc.vector.tensor_copy(out=dst3, in_=src3)
                    done.add(f"w4_{m}")
                ex_q.append(fin)

            # order matters: consumers force-pop up to their marker
            push_1a_half(0, 1, 0)
            push_1a_half(0, 1, 1)
            for m in range(MCH):
                push_ph2(m)
            push_1b_half(0, 1, 0)
            push_1b_half(0, 1, 1, marker="std_hp0_n1")
            push_1a_half(1, 0, 0)
            push_1a_half(1, 0, 1)
            push_1a_half(1, 1, 0)
            push_1a_half(1, 1, 1)
            push_1b_half(1, 0, 0)
            push_1b_half(1, 0, 1, marker="std_hp1_n0")
            push_1b_half(1, 1, 0)
            push_1b_half(1, 1, 1, marker="std_hp1_n1")

            def pop_extras(k):
                while k > 0 and qi[0] < len(ex_q):
                    ex_q[qi[0]]()
                    qi[0] += 1
                    k -= 1

            def force(marker):
                while marker not in done and qi[0] < len(ex_q):
                    ex_q[qi[0]]()
                    qi[0] += 1

            # ---- misc queue (norms + mid out-proj), popped 1 per step ----
            misc_q = []

            def pop_misc():
                if misc_q:
                    misc_q.pop(0)()

            def push_norm(opair, use_pps=False):
                """normalize the (opair-1, opair) head-pair's uta block."""
                s = (opair // 2) % 2
                nbp, j = PAIR_SEQ[opair]
                hp = j // 2

                def t_recip(s=s):
                    nc.vector.reciprocal_approx_fast(out=inv0[s], in_=csr[s])
                    nc.vector.tensor_copy(out=invr[s], in_=inv0[s])
                misc_q.append(t_recip)
                for h in range(2):
                    def t_bc(s=s, nbp=nbp, hp=hp, h=h, use_pps=use_pps):
                        if use_pps:
                            pb = pps.tile([128, 1024], F32, tag="ps",
                                          name="pb")[:, 0:512]
                        else:
                            pb = ppo.tile([128, 512], F32, tag="po",
                                          name="pb")
                        nc.tensor.matmul(
                            pb, ones2sel,
                            invr[s][:, h * 512:(h + 1) * 512],
                            start=True, stop=True)
                        nsl = slice(nbp * 1024 + h * 512,
                                    nbp * 1024 + (h + 1) * 512)
                        nc.vector.tensor_mul(uta[hp][:, nsl],
                                             uta[hp][:, nsl], pb)
                    misc_q.append(t_bc)

            osb_cache = {}

            def push_outproj(nbp):
                for nch in range(nbp * 8, nbp * 8 + 8):
                    for qb in range(2):
                        def t_op(nch=nch, qb=qb):
                            po = ppo.tile([128, 512], F32, tag="po",
                                          name="po")
                            for j2 in range(2):
                                nc.tensor.matmul(
                                    po,
                                    uta[j2][:, nch * 128:(nch + 1) * 128],
                                    thyt(j2, qb * 512, (qb + 1) * 512),
                                    start=(j2 == 0), stop=(j2 == 1))
                            if qb == 0:
                                osb_cache[nch] = op.tile(
                                    [128, 1024], BF16, tag="osb", bufs=4,
                                    name="osb")
                            osb = osb_cache[nch]
                            nc.vector.tensor_copy(
                                out=osb[:, qb * 512:(qb + 1) * 512], in_=po)
                            if qb == 1:
                                nc.gpsimd.dma_start(
                                    out=outp[nch * 128:(nch + 1) * 128, :],
                                    in_=osb)
                        misc_q.append(t_op)

            # ---- emit helpers ----
            e_t = {}

            def emit_scores(pair, m):
                nbp, j = PAIR_SEQ[pair]
                hp, n0 = j // 2, nbp * 1024
                force(f"r1T{hp}_{m // 8}_{(m % 8) // 4}")
                force(f"std_hp{hp}_n{nbp}")
                pst = pps.tile([128, 1024], F32, tag="ps", name="ps")
                for h in range(2):
                    nc.tensor.matmul(
                        pst[:, h * 512:(h + 1) * 512],
                        r1T[hp][:, m * 128:(m + 1) * 128],
                        std[j][:, n0 + h * 512:n0 + (h + 1) * 512],
                        start=True, stop=True)
                if use_mask:
                    mt = op.tile([128, 1024], F32, tag="mt", name="mt")
                    nc.gpsimd.dma_start(
                        out=mt, in_=maskd[m * 128:(m + 1) * 128, n0:n0 + 1024])
                    nc.vector.tensor_add(pst, pst, mt)
                et = ep.tile([128, 1024], BF16, tag="e", name="e")
                e_t[(pair, m)] = et
                nc.scalar.activation(
                    out=et, in_=pst, func=mybir.ActivationFunctionType.Exp,
                    scale=INV_TEMP)

            def emit_agg(pair, ms, pu):
                j = PAIR_SEQ[pair][1]
                for m in ms:
                    force(f"w4_{m}")
                    for h in range(2):
                        nc.tensor.matmul(
                            pu[:, h * 512:(h + 1) * 512],
                            w4[m][:, j * 65:j * 65 + 65],
                            e_t[(pair, m)][:, h * 512:(h + 1) * 512],
                            start=(m == 0), stop=(m == MCH - 1))
                    e_t.pop((pair, m))

            def emit_drain(pair, pu):
                nbp, j = PAIR_SEQ[pair]
                hp, po, n0 = j // 2, 64 * (j % 2), nbp * 1024
                s, row = (pair // 2) % 2, 32 * (pair % 2)
                nc.vector.tensor_copy(out=uta[hp][po:po + 64, n0:n0 + 1024],
                                      in_=pu[0:64, :])
                nc.vector.tensor_copy(out=csr[s][row:row + 1, :],
                                      in_=pu[64:65, :])

            # ---- windows ----
            pu_cur = [None]     # pair w-1's [65,1024] pu (ppu pool)
            pu7h = [None, None]  # pair 7 trailing pu halves (ppo slices)
            for w in range(NPAIR):
                nbp, j = PAIR_SEQ[w]
                for step in range(16):
                    if w == NPAIR - 1 and step == 0:
                        # ppo must be free for the trailing pu halves
                        pop_extras(len(ex_q))
                        while misc_q:
                            pop_misc()
                    has_agg = False
                    if w >= 1 and step < len(AGG_MS):
                        if step == 0:
                            pu_cur[0] = ppu.tile([65, 1024], F32, tag="pu",
                                                 name="pu")
                        emit_agg(w - 1, AGG_MS[step], pu_cur[0])
                        has_agg = True
                    emit_scores(w, step)
                    if w == NPAIR - 1 and step >= 2:
                        # trailing aggregation of the last pair
                        m = step - 2
                        if m == 0:
                            pu7h[0] = ppo.tile([128, 512], F32, tag="po",
                                               name="pu7a")[0:65, :]
                            pu7h[1] = ppo.tile([128, 512], F32, tag="po",
                                               name="pu7b")[0:65, :]
                        force(f"w4_{m}")
                        for h in range(2):
                            nc.tensor.matmul(
                                pu7h[h],
                                w4[m][:, j * 65:j * 65 + 65],
                                e_t[(w, m)][:, h * 512:(h + 1) * 512],
                                start=(m == 0), stop=(m == MCH - 1))
                        e_t.pop((w, m))
                    if w >= 1 and step == 14:
                        emit_drain(w - 1, pu_cur[0])
                    if w >= 3:
                        pop_misc()
                    if step >= 14:
                        pop_extras(2)
                    elif has_agg and len(AGG_MS[step]) > 1:
                        pass  # double-agg slots are already over budget
                    else:
                        pop_extras(3 if not has_agg else (2 if w < 4 else 1))
                # norm thunks become available after the odd pair's drain
                if w >= 1 and w - 1 <= 5 and (w - 1) % 2 == 1:
                    push_norm(w - 1)
                    if w - 1 == 5:
                        push_outproj(0)

            # ---- ghost tail ----
            pop_extras(len(ex_q))
            while misc_q:
                pop_misc()
            # pair 7 trailing remnant: m=14,15 (steps 2..15 covered 0..13)
            j7 = (NPAIR - 1) % HPC
            for m in range(14, MCH):
                for h in range(2):
                    nc.tensor.matmul(
                        pu7h[h],
                        w4[m][:, j7 * 65:j7 * 65 + 65],
                        e_t[(NPAIR - 1, m)][:, h * 512:(h + 1) * 512],
                        start=(m == 0), stop=(m == MCH - 1))
                e_t.pop((NPAIR - 1, m))
            # --- ghost norm + out-projection, pipelined by n-half ---
            # pairs 6,7 share colsum set 1: pair 6's row 0 was written by
            # its w7.14 drain; pair 7's row 32 comes from the pu7h halves
            for h in range(2):
                nc.vector.tensor_copy(
                    out=csr[1][32:33, h * 512:(h + 1) * 512],
                    in_=pu7h[h][64:65, :])
            nc.vector.reciprocal_approx_fast(out=inv0[1], in_=csr[1])
            nc.vector.tensor_copy(out=invr[1], in_=inv0[1])
            emit_dummies(6, pps)
            # pair-7 uta rows drain on the idle ACT
            for h in range(2):
                sl = slice(1024 + h * 512, 1024 + (h + 1) * 512)
                nc.scalar.copy(out=uta[1][64:128, sl], in_=pu7h[h][0:64, :])

            def ghost_norm(h):
                pb = pps.tile([128, 1024], F32, tag="ps",
                              name="pb")[:, 0:512]
                nc.tensor.matmul(pb, ones2sel,
                                 invr[1][:, h * 512:(h + 1) * 512],
                                 start=True, stop=True)
                nsl = slice(1024 + h * 512, 1024 + (h + 1) * 512)
                nc.vector.tensor_mul(uta[1][:, nsl], uta[1][:, nsl], pb)

            for h in range(2):
                ghost_norm(h)
                if h == 0:
                    emit_dummies(2, pps)
                # out-projection for this n-half (nch 8+4h .. 11+4h)
                for i, (nch, qb) in enumerate(
                        (nc_, qb_) for nc_ in range(8 + 4 * h, 12 + 4 * h)
                        for qb_ in range(2)):
                    po = pps.tile([128, 1024], F32, tag="ps",
                                  name="pof")[:, 0:512]
                    for j2 in range(2):
                        nc.tensor.matmul(
                            po,
                            uta[j2][:, nch * 128:(nch + 1) * 128],
                            thyt(j2, qb * 512, (qb + 1) * 512),
                            start=(j2 == 0), stop=(j2 == 1))
                    if qb == 0:
                        osb_cache[nch] = op.tile([128, 1024], BF16,
                                                 tag="osb", bufs=4,
                                                 name="osbf")
                    osb = osb_cache[nch]
                    if i % 2 == 0:
                        nc.vector.tensor_copy(
                            out=osb[:, qb * 512:(qb + 1) * 512], in_=po)
                    else:
                        nc.scalar.copy(out=osb[:, qb * 512:(qb + 1) * 512],
                                       in_=po)
                    if qb == 1:
                        dq = (nc.gpsimd, nc.sync, nc.scalar)[nch % 3]
                        dq.dma_start(
                            out=outp[nch * 128:(nch + 1) * 128, :], in_=osb)
    lp.__exit__(None, None, None)
    nc.compile()
    return nc


_PROG_CACHE = {}


def _get_program(use_mask: bool):
    if use_mask not in _PROG_CACHE:
        _PROG_CACHE[use_mask] = build_program(use_mask)
    return _PROG_CACHE[use_mask]


def _pack(a):
    """[(c*128), k] -> [128, c*k]: p-chunks side by side, SBUF layout."""
    c = a.shape[0] // 128
    return np.ascontiguousarray(
        a.reshape(c, 128, a.shape[1]).transpose(1, 0, 2).reshape(128, -1)
    ).astype(ml_dtypes.bfloat16)


def make_in_maps(y_prime, mask, Lam_x, Lam_y, Th_x, Th_y, use_mask):
    in_maps = []
    for c in range(NCORES):
        b = c // 4
        heads = [4 * (c % 4) + j for j in range(HPC)]
        m = {
            "yT": _pack(y_prime[b].T),
            "lamx": _pack(Lam_x[heads].transpose(1, 0, 2).reshape(P, HPC * D)),
            "lamy": _pack(Lam_y[heads].transpose(1, 0, 2).reshape(P, HPC * D)),
            "thx": _pack(Th_x[heads].transpose(1, 0, 2).reshape(P, HPC * D)),
            "thyT": _pack(Th_y[heads].transpose(0, 2, 1).reshape(HPC * D, Q)),
        }
        if use_mask:
            m["mask"] = np.ascontiguousarray(mask).astype(np.float32)
        in_maps.append(m)
    return in_maps


def kernel(y_prime, mask, Lam_x, Lam_y, Th_x, Th_y, _trace=False):
    y_prime = np.asarray(y_prime, dtype=np.float32)
    mask = np.asarray(mask, dtype=np.float32)
    Lam_x = np.asarray(Lam_x, dtype=np.float32)
    Lam_y = np.asarray(Lam_y, dtype=np.float32)
    Th_x = np.asarray(Th_x, dtype=np.float32)
    Th_y = np.asarray(Th_y, dtype=np.float32)

    use_mask = bool(np.any(mask))
    nc = _get_program(use_mask)
    in_maps = make_in_maps(y_prime, mask, Lam_x, Lam_y, Th_x, Th_y, use_mask)
    r = run_bass_kernel_spmd(nc, in_maps, core_ids=list(range(NCORES)),
                             trace=_trace)
    out = np.zeros((B, N, Q), dtype=np.float32)
    for c in range(NCORES):
        out[c // 4] += np.asarray(r.results[c]["outp"]).astype(np.float32)
    if _trace:
        kernel.last_results = r
    return out
